# revision 1
# baseline (speedup 1.0000x reference)
"""Trainium2 8-core Bass kernel for the SKalmanNet dense-MLP GEMV chain.

Network (batch=1):
  x   = concat(state_inno, precov, residual, meas_cov)          [128]
  l1  = relu(W1 @ x + b1)                                       [1344]
  gi  = w_ih @ l1 + b_ih ; gh = w_hh @ h0 + b_hh                [12288]
  r,z = sigmoid(gi+gh) gates ; n = tanh(gi_n + r*gh_n)
  h   = (1-z)*n + z*h0                                          [4096]
  x_hat = W2b @ relu(W2a @ h + b2a) + b2b                       [32]
  P_hat = W3b @ relu(W3a @ h + b3a) + b3b                       [32]

Sharding: every large matrix is row-sharded (output dim) across 8 cores;
W1 is replicated (tiny) so l1 needs no collective. The only collective is
one 16KB AllGather of h. The final 32-vector partials (W2b/W3b column
shards) are summed on the host during unsharding.

Layouts: activations live as "stationary" columns [128, nblk] so they can
be the matmul lhsT; weights are host-pre-transposed so W.T tiles stream
as the rhs. All biases are folded into the matmuls via an augmented
contraction element that is constant 1.
"""

import os
import sys

sys.path.insert(0, "/opt/trn_rl_repo")

import numpy as np
import ml_dtypes

# ---------------------------------------------------------------- constants
NCORES = 8
X_DIM = 32
IN2 = 128                      # l1 input dim
H1 = 1344                      # l1 output / GRU input dim
H1P = 1408                     # padded to 11*128 (pad block holds the bias row)
GH = 4096                      # GRU hidden
GHP = 4224                     # padded to 33*128 (aug block holds bias row)
H2 = 4096                      # head hidden
SH = 512                       # per-core hidden slice (GH/8 == H2/8)
K1 = H1P // 128                # 11 contraction blocks for gi
KH = GHP // 128                # 33 contraction blocks for gh / heads
KF = 640 // 128                # 5 contraction blocks for the final gemv

GRU_CHUNK = 4                  # k-blocks per DMA chunk for wiht/whht
HEAD_CHUNK = 8                 # k-blocks per DMA chunk for w2at/w3at

_WDT_NAME = os.environ.get("KERNEL_DTYPE", "bf16")
_GATHER = os.environ.get("KERNEL_GATHER", "cc")

_compiled = {}


def _np_wdt():
    return {"bf16": ml_dtypes.bfloat16, "f32": np.float32, "f32r": np.float32}[
        _WDT_NAME
    ]


def _build(wdt_name, gather):
    import concourse.bass as bass  # noqa: F401
    import concourse.mybir as mybir
    import concourse.tile as tile
    from concourse import bacc

    F32 = mybir.dt.float32
    WDT = {
        "bf16": mybir.dt.bfloat16,
        "f32": mybir.dt.float32,
        "f32r": mybir.dt.float32r,
    }[wdt_name]
    AF = mybir.ActivationFunctionType
    ALU = mybir.AluOpType
    ts = bass.ts

    nc = bacc.Bacc("TRN2", target_bir_lowering=False, debug=False, num_devices=NCORES)

    # ------------------------------------------------------------- I/O decl
    xvec = nc.dram_tensor("xvec", [128, 1], WDT, kind="ExternalInput")
    w1t = nc.dram_tensor("w1t", [128, H1P], WDT, kind="ExternalInput")
    b1s = nc.dram_tensor("b1s", [128, K1], F32, kind="ExternalInput")
    # packed streams: per chunk, [128, nkb*N] with per-partition-contiguous
    # bytes so each DMA descriptor is one long run
    wihp = nc.dram_tensor("wihp", [K1 * 128 * 3 * SH], WDT, kind="ExternalInput")
    whhp = nc.dram_tensor("whhp", [KH * 128 * 3 * SH], WDT, kind="ExternalInput")
    w2ap = nc.dram_tensor("w2ap", [KH * 128 * SH], WDT, kind="ExternalInput")
    w3ap = nc.dram_tensor("w3ap", [KH * 128 * SH], WDT, kind="ExternalInput")
    h0stat = nc.dram_tensor("h0stat", [128, KH], WDT, kind="ExternalInput")
    h0row = nc.dram_tensor("h0row", [1, SH], F32, kind="ExternalInput")
    w2bt = nc.dram_tensor("w2bt", [KF * 128, 32], WDT, kind="ExternalInput")
    w3bt = nc.dram_tensor("w3bt", [KF * 128, 32], WDT, kind="ExternalInput")
    ident = nc.dram_tensor("ident", [32, 128], F32, kind="ExternalInput")
    coreid = nc.dram_tensor("coreid", [1, 1], mybir.dt.uint32, kind="ExternalInput")
    out = nc.dram_tensor("out", [1, 64], F32, kind="ExternalOutput")

    def chunks_of(t, nkb_total, step, width):
        out = []
        for kb0 in range(0, nkb_total, step):
            out.append((t, kb0, min(step, nkb_total - kb0), width))
        return out

    gru_chunks = chunks_of(wihp, K1, GRU_CHUNK, 3 * SH) + chunks_of(
        whhp, KH, GRU_CHUNK, 3 * SH
    )
    n_wih_chunks = len(chunks_of(wihp, K1, GRU_CHUNK, 3 * SH))
    head_chunks = chunks_of(w2ap, KH, HEAD_CHUNK, SH) + chunks_of(
        w3ap, KH, HEAD_CHUNK, SH
    )
    # last 3 full w3at chunks stream on the ACT ring, FIFO-gated behind the
    # gather output load, so the fabric is quiet while the collective runs
    N_LATE = 0

    with tile.TileContext(nc) as tc:
        with (
            tc.tile_pool(name="const", bufs=1) as cp,
            tc.tile_pool(name="gru", bufs=4) as gp,
            tc.tile_pool(name="head", bufs=10) as hp,
            tc.tile_pool(name="acts", bufs=1) as ap,
            tc.tile_pool(name="dram", bufs=1, space="DRAM") as dp,
        ):
            # ------------------------------------------------ constant loads
            x_sb = cp.tile([128, 1], WDT, tag="x")
            nc.sync.dma_start(x_sb[:], xvec[:])
            w1_sb = cp.tile([128, H1P], WDT, tag="w1")
            nc.sync.dma_start(w1_sb[:], w1t[:])
            b1_sb = cp.tile([128, K1], F32, tag="b1")
            nc.sync.dma_start(b1_sb[:], b1s[:])
            h0s_sb = cp.tile([128, KH], WDT, tag="h0s")
            nc.sync.dma_start(h0s_sb[:], h0stat[:])
            h0r_sb = cp.tile([1, SH], F32, tag="h0r")
            nc.sync.dma_start(h0r_sb[:], h0row[:])
            id_sb = cp.tile([32, 128], F32, tag="id")
            nc.sync.dma_start(id_sb[:], ident[:])
            cid_sb = cp.tile([1, 1], mybir.dt.uint32, tag="cid")
            nc.sync.dma_start(cid_sb[:], coreid[:])
            # gather target: written remotely by all 8 cores' broadcasts.
            # memset early so the slot is reserved for the whole kernel and
            # cannot alias a streaming tile when a peer's write lands.
            h_sb = ap.tile([128, KH], WDT, tag="hstat")
            nc.vector.memset(h_sb[:], 0.0)
            w2b_sb = cp.tile([128, KF, 32], WDT, tag="w2b")
            nc.sync.dma_start(
                w2b_sb[:], w2bt[:].rearrange("(k p) n -> p k n", p=128)
            )
            w3b_sb = cp.tile([128, KF, 32], WDT, tag="w3b")
            nc.sync.dma_start(
                w3b_sb[:], w3bt[:].rearrange("(k p) n -> p k n", p=128)
            )

            # startup all-core rendezvous on gpsimd only: absorbs per-core
            # NEFF launch skew (tens of us) off the critical path, while the
            # SP ring streams weights unaffected. Also warms ncfw for the
            # real gather.
            bar_sb = cp.tile([1, 8], mybir.dt.uint32, tag="bar")
            nc.gpsimd.memset(bar_sb[:], 1)
            bar_in = dp.tile([1, 8], mybir.dt.uint32, name="bar_in")
            bar_out = dp.tile([1, 8], mybir.dt.uint32, name="bar_out")
            nc.gpsimd.dma_start(bar_in[:], bar_sb[:])
            nc.gpsimd.collective_compute(
                "AllReduce",
                mybir.AluOpType.add,
                replica_groups=[list(range(NCORES))],
                ins=[bar_in[:].opt()],
                outs=[bar_out[:].opt()],
            )
            bar2_sb = cp.tile([1, 8], mybir.dt.uint32, tag="bar2")
            nc.gpsimd.dma_start(bar2_sb[:], bar_out[:])

            # ------------------------------------- weight stream DMAs (HWDGE)
            def stream_chunk(pool, spec, tag, engine):
                t, kb0, nkb, width = spec
                g = pool.tile([128, GRU_CHUNK * 3 * SH] if width == 3 * SH
                              else [128, HEAD_CHUNK * SH], WDT, tag=tag, name=tag)
                off = kb0 * 128 * width
                sz = nkb * 128 * width
                src_ap = t[off : off + sz].rearrange("(p x) -> p x", p=128)
                engine.dma_start(g[:, 0 : nkb * width], src_ap)
                return g

            gru_tiles = []
            for spec in gru_chunks[:n_wih_chunks]:
                gru_tiles.append(stream_chunk(gp, spec, "gruw", nc.sync))
            head_tiles = []
            for spec in head_chunks[: len(head_chunks) - N_LATE]:
                head_tiles.append(stream_chunk(hp, spec, "headw", nc.sync))
            for spec in gru_chunks[n_wih_chunks:]:
                gru_tiles.append(stream_chunk(gp, spec, "gruw", nc.sync))

            with tc.tile_pool(name="psA", bufs=1, space="PSUM") as psA:
                # ------------------------------------------- L1 (W-stationary)
                l1p = psA.tile([128, K1], F32, tag="l1p")
                for j in range(K1):
                    nc.tensor.matmul(
                        l1p[:, j : j + 1],
                        w1_sb[:, ts(j, 128)],
                        x_sb[:],
                        start=True,
                        stop=True,
                    )
                l1t = ap.tile([128, K1], F32, tag="l1t")
                nc.vector.scalar_tensor_tensor(
                    l1t[:], l1p[:], 1.0, b1_sb[:], ALU.mult, ALU.add
                )
                l1_sb = ap.tile([128, K1], WDT, tag="l1s")
                nc.scalar.activation(l1_sb[:], l1t[:], AF.Relu)

                # ------------------------------------------- GRU matmuls
                gi = [psA.tile([1, SH], F32, tag=f"gi{g}", name=f"gi{g}") for g in range(3)]
                gh = [psA.tile([1, SH], F32, tag=f"gh{g}", name=f"gh{g}") for g in range(3)]
                for ci, (t, kb0, nkb, width) in enumerate(gru_chunks):
                    is_ih = ci < n_wih_chunks
                    dst = gi if is_ih else gh
                    stat = l1_sb if is_ih else h0s_sb
                    klast = (K1 if is_ih else KH) - 1
                    for kk in range(nkb):
                        kb = kb0 + kk
                        for g in range(3):
                            base = kk * width + g * SH
                            nc.tensor.matmul(
                                dst[g][:],
                                stat[:, kb : kb + 1],
                                gru_tiles[ci][:, base : base + SH],
                                start=(kb == 0),
                                stop=(kb == klast),
                            )

                # gi -> SBUF (ScalarE; overlaps the gh matmul stream). DVE has
                # a single PSUM read port, so gate ops may touch <=1 PSUM operand.
                gis = ap.tile([1, 3 * SH], F32, tag="gis")
                for g in range(3):
                    nc.scalar.activation(gis[:, ts(g, SH)], gi[g][:], AF.Copy)

                # ------------------------------------------- gates (row layout)
                t_r = ap.tile([1, SH], F32, tag="gtmp", bufs=6)
                nc.vector.tensor_tensor(t_r[:], gis[:, ts(0, SH)], gh[0][:], ALU.add)
                r = ap.tile([1, SH], F32, tag="r")
                nc.scalar.activation(r[:], t_r[:], AF.Sigmoid)
                t_z = ap.tile([1, SH], F32, tag="gtmp", bufs=6)
                nc.vector.tensor_tensor(t_z[:], gis[:, ts(1, SH)], gh[1][:], ALU.add)
                z = ap.tile([1, SH], F32, tag="z")
                nc.scalar.activation(z[:], t_z[:], AF.Sigmoid)
                t_m = ap.tile([1, SH], F32, tag="gtmp", bufs=6)
                nc.vector.tensor_tensor(t_m[:], r[:], gh[2][:], ALU.mult)
                t_n = ap.tile([1, SH], F32, tag="gtmp", bufs=6)
                nc.vector.tensor_tensor(t_n[:], t_m[:], gis[:, ts(2, SH)], ALU.add)
                n_t = ap.tile([1, SH], F32, tag="n")
                nc.scalar.activation(n_t[:], t_n[:], AF.Tanh)
                t_d = ap.tile([1, SH], F32, tag="gtmp", bufs=6)
                nc.vector.tensor_tensor(t_d[:], h0r_sb[:], n_t[:], ALU.subtract)
                t_e = ap.tile([1, SH], F32, tag="gtmp", bufs=6)
                nc.vector.tensor_tensor(t_e[:], z[:], t_d[:], ALU.mult)
                h_row = ap.tile([1, SH], F32, tag="hrow")
                nc.vector.tensor_tensor(h_row[:], n_t[:], t_e[:], ALU.add)

            # ------------- h row -> stationary cols via rank-1 PE matmuls
            one = id_sb[0:1, 0:1]  # constant 1.0
            with tc.tile_pool(name="psB", bufs=1, space="PSUM") as psB:
                hT4 = psB.tile([128, 4], F32, tag="hT4")
                for k in range(4):
                    nc.tensor.matmul(
                        hT4[:, k : k + 1],
                        h_row[0:1, ts(k, 128)],
                        one,
                        start=True,
                        stop=True,
                    )
                hloc = ap.tile([128, 4], WDT, tag="hloc")
                nc.vector.tensor_copy(hloc[:], hT4[:])

                # ---------------- all-gather h across the 8 cores
                h_use = ap.tile([128, KH], WDT, tag="huse")
                if gather == "bcast":
                    psem = nc.alloc_semaphore("bc_prep_sem")
                    lsem = nc.alloc_semaphore("bc_local_sem")
                    rsem = nc.alloc_semaphore("bc_remote_sem")
                    with tc.tile_critical():
                        eng = nc.gpsimd
                        reg = eng.alloc_register("cid_reg")
                        # order after the startup alignment barrier
                        eng.reg_load(reg, bar2_sb[0:1, 0:1])
                        eng.reg_load(reg, cid_sb[0:1, 0:1])
                        for c in range(NCORES):
                            with eng.If_eq(reg, c):
                                eng.remote_dma_broadcast(
                                    out_ap=h_sb[:, c * 4 : (c + 1) * 4],
                                    in_ap=hloc[:],
                                    remote_sem=rsem,
                                    local_sem=lsem,
                                    rdests=[(0, k) for k in range(NCORES)],
                                ).then_inc(psem, 1)
                            with eng.Else():
                                eng.nop()
                        eng.wait_ge(psem, 1)
                        eng.trigger_dma(count=1)
                        eng.wait_ge(lsem, 16)
                        eng.wait_ge(rsem, 16)
                        eng.memset(h_sb[0:1, 32:33], 1.0)  # aug element
                        # copy into h_use so downstream consumers depend on
                        # the gathered data (remote writes invisible to Tile)
                        eng.tensor_copy(h_use[:], h_sb[:])
                else:
                    cc_in = dp.tile([128, 4], WDT, name="cc_in")
                    cc_out = dp.tile([NCORES, 128, 4], WDT, name="cc_out")
                    nc.scalar.dma_start(cc_in[:], hloc[:])
                    nc.gpsimd.collective_compute(
                        "AllGather",
                        mybir.AluOpType.bypass,
                        replica_groups=[list(range(NCORES))],
                        ins=[cc_in[:].opt()],
                        outs=[cc_out[:].opt()],
                    )
                    # cc_out[c, p, j] = h block col (c*4+j) partition p
                    nc.scalar.dma_start(
                        h_sb[:, 0:32].rearrange("p (c j) -> p c j", j=4),
                        cc_out[:].rearrange("c p j -> p c j"),
                    )
                    nc.vector.memset(h_sb[0:1, 32:33], 1.0)
                    nc.vector.tensor_copy(h_use[:], h_sb[:])
                for spec in head_chunks[len(head_chunks) - N_LATE :]:
                    head_tiles.append(stream_chunk(hp, spec, "headw", nc.scalar))

                # ------------------------------------------- head matmuls
                a2p = psB.tile([1, SH], F32, tag="a2p")
                a3p = psB.tile([1, SH], F32, tag="a3p")
                nh = len(head_chunks) // 2
                for ci, (t, kb0, nkb, width) in enumerate(head_chunks):
                    dst = a2p if ci < nh else a3p
                    for kk in range(nkb):
                        kb = kb0 + kk
                        nc.tensor.matmul(
                            dst[:],
                            h_use[:, kb : kb + 1],
                            head_tiles[ci][:, kk * SH : (kk + 1) * SH],
                            start=(kb == 0),
                            stop=(kb == KH - 1),
                        )

                a2row = ap.tile([1, SH], F32, tag="a2row")
                nc.scalar.activation(a2row[:], a2p[:], AF.Relu)
                a3row = ap.tile([1, SH], F32, tag="a3row")
                nc.scalar.activation(a3row[:], a3p[:], AF.Relu)

                # ---------------- a rows -> stationary cols (rank-1 PE)
                aT2 = psB.tile([128, 4], F32, tag="aT2")
                aT3 = psB.tile([128, 4], F32, tag="aT3")
                for k in range(4):
                    nc.tensor.matmul(
                        aT2[:, k : k + 1], a2row[0:1, ts(k, 128)], one,
                        start=True, stop=True,
                    )
                for k in range(4):
                    nc.tensor.matmul(
                        aT3[:, k : k + 1], a3row[0:1, ts(k, 128)], one,
                        start=True, stop=True,
                    )
                a_sb = ap.tile([128, 9], WDT, tag="astat")
                nc.vector.tensor_copy(a_sb[:, 0:4], aT2[:])
                nc.vector.tensor_copy(a_sb[:, 4:8], aT3[:])
                nc.vector.memset(a_sb[:, 8:9], 0.0)
                nc.vector.memset(a_sb[0:1, 8:9], 1.0)

                # ------------------------------------------- final gemvs
                op = psB.tile([1, 64], F32, tag="outp")
                cols2 = [0, 1, 2, 3, 8]
                cols3 = [4, 5, 6, 7, 8]
                for ki, k in enumerate(cols2):
                    nc.tensor.matmul(
                        op[:, 0:32],
                        a_sb[:, k : k + 1],
                        w2b_sb[:, ki, :],
                        start=(ki == 0),
                        stop=(ki == KF - 1),
                    )
                for ki, k in enumerate(cols3):
                    nc.tensor.matmul(
                        op[:, 32:64],
                        a_sb[:, k : k + 1],
                        w3b_sb[:, ki, :],
                        start=(ki == 0),
                        stop=(ki == KF - 1),
                    )
                out_sb = ap.tile([1, 64], F32, tag="osb")
                nc.scalar.activation(out_sb[:], op[:], AF.Copy)
                nc.gpsimd.dma_start(out[:], out_sb[:])

    nc.compile()
    return nc


def _get_nc():
    if _WDT_NAME not in _compiled:
        _compiled[_WDT_NAME] = _build(_WDT_NAME, _GATHER)
    return _compiled[_WDT_NAME]


# ------------------------------------------------------------------ host prep
def _prep_in_maps(inputs):
    wnp = _np_wdt()
    f32 = np.float32

    def W(a):
        return np.ascontiguousarray(a, dtype=np.float32).astype(wnp)

    x = np.concatenate(
        [
            np.asarray(inputs[k], dtype=f32).ravel()
            for k in ("state_inno", "precov", "residual", "meas_cov")
        ]
    )
    W1 = np.asarray(inputs["W1"], f32)
    b1 = np.asarray(inputs["b1"], f32)
    w_ih = np.asarray(inputs["w_ih"], f32)
    w_hh = np.asarray(inputs["w_hh"], f32)
    b_ih = np.asarray(inputs["b_ih"], f32)
    b_hh = np.asarray(inputs["b_hh"], f32)
    h0 = np.asarray(inputs["h0"], f32)
    W2a = np.asarray(inputs["W2a"], f32)
    b2a = np.asarray(inputs["b2a"], f32)
    W2b = np.asarray(inputs["W2b"], f32)
    b2b = np.asarray(inputs["b2b"], f32)
    W3a = np.asarray(inputs["W3a"], f32)
    b3a = np.asarray(inputs["b3a"], f32)
    W3b = np.asarray(inputs["W3b"], f32)
    b3b = np.asarray(inputs["b3b"], f32)

    # shared (core-independent) tensors
    xvec = W(x).reshape(128, 1)
    w1t = np.zeros((128, H1P), f32)
    w1t[:, :H1] = W1.T
    w1t = w1t.astype(wnp)
    b1s = np.zeros((128, K1), f32)
    b1pad = np.zeros(H1P, f32)
    b1pad[:H1] = b1
    b1pad[H1] = 1.0  # aug element: relu(0 + 1) = 1 feeds the bias rows of gi
    b1s[:, :] = b1pad.reshape(K1, 128).T
    h0stat = np.zeros((128, KH), f32)
    h0stat[:, :32] = h0.reshape(32, 128).T
    h0stat[0, 32] = 1.0
    h0stat = h0stat.astype(wnp)
    identity = np.zeros((32, 128), dtype=f32)
    identity[:, :32] = np.eye(32, dtype=f32)

    wihT = w_ih.T  # [H1, 3GH]
    whhT = w_hh.T  # [GH, 3GH]
    W2aT = W2a.T  # [GH, H2]
    W3aT = W3a.T

    in_maps = []
    for c in range(NCORES):
        s = slice(c * SH, (c + 1) * SH)
        gcols = np.r_[np.arange(c * SH, (c + 1) * SH),
                      np.arange(GH + c * SH, GH + (c + 1) * SH),
                      np.arange(2 * GH + c * SH, 2 * GH + (c + 1) * SH)]

        wiht = np.zeros((H1P, 3 * SH), f32)
        wiht[:H1, :] = wihT[:, gcols]
        wiht[H1, :] = b_ih[gcols]
        whht = np.zeros((GHP, 3 * SH), f32)
        whht[:GH, :] = whhT[:, gcols]
        whht[GH, :] = b_hh[gcols]
        w2at = np.zeros((GHP, SH), f32)
        w2at[:GH, :] = W2aT[:, s]
        w2at[GH, :] = b2a[s]
        w3at = np.zeros((GHP, SH), f32)
        w3at[:GH, :] = W3aT[:, s]
        w3at[GH, :] = b3a[s]

        def pack_stream(mat, nkb_total, step):
            width = mat.shape[1]
            blocks = []
            for kb0 in range(0, nkb_total, step):
                nkb = min(step, nkb_total - kb0)
                blk = (
                    mat[kb0 * 128 : (kb0 + nkb) * 128, :]
                    .reshape(nkb, 128, width)
                    .transpose(1, 0, 2)
                    .reshape(-1)
                )
                blocks.append(blk)
            return np.concatenate(blocks)

        wihp = pack_stream(wiht, K1, GRU_CHUNK)
        whhp = pack_stream(whht, KH, GRU_CHUNK)
        w2ap = pack_stream(w2at, KH, HEAD_CHUNK)
        w3ap = pack_stream(w3at, KH, HEAD_CHUNK)
        w2bt = np.zeros((KF * 128, 32), f32)
        w2bt[:SH, :] = W2b[:, s].T
        w3bt = np.zeros((KF * 128, 32), f32)
        w3bt[:SH, :] = W3b[:, s].T
        if c == 0:
            w2bt[SH, :] = b2b
            w3bt[SH, :] = b3b

        in_maps.append(
            {
                "xvec": xvec,
                "w1t": w1t,
                "b1s": b1s,
                "wihp": wihp.astype(wnp),
                "whhp": whhp.astype(wnp),
                "h0stat": h0stat,
                "h0row": h0[s].reshape(1, SH),
                "w2ap": w2ap.astype(wnp),
                "w3ap": w3ap.astype(wnp),
                "w2bt": w2bt.astype(wnp),
                "w3bt": w3bt.astype(wnp),
                "ident": identity,
                "coreid": np.array([[c]], dtype=np.uint32),
            }
        )
    return in_maps


def run(inputs, trace=False):
    from concourse.bass_utils import run_bass_kernel_spmd

    nc = _get_nc()
    in_maps = _prep_in_maps(inputs)
    res = run_bass_kernel_spmd(
        nc, in_maps, core_ids=list(range(NCORES)), trace=trace
    )
    total = np.sum([np.asarray(r["out"], np.float64) for r in res.results], axis=0)
    total = total.astype(np.float32).ravel()
    x_hat = total[:32].reshape(X_DIM, 1)
    P_hat = total[32:].reshape(X_DIM, 1)
    return (x_hat, P_hat), res


def kernel(**inputs):
    (x_hat, P_hat), _ = run(inputs, trace=False)
    return (x_hat, P_hat)



# revision 2
# speedup vs baseline: 1.1775x; 1.1775x over previous
"""Trainium2 8-core Bass kernel for the SKalmanNet dense-MLP GEMV chain.

Network (batch=1):
  x   = concat(state_inno, precov, residual, meas_cov)          [128]
  l1  = relu(W1 @ x + b1)                                       [1344]
  gi  = w_ih @ l1 + b_ih ; gh = w_hh @ h0 + b_hh                [12288]
  r,z = sigmoid(gi+gh) gates ; n = tanh(gi_n + r*gh_n)
  h   = (1-z)*n + z*h0                                          [4096]
  x_hat = W2b @ relu(W2a @ h + b2a) + b2b                       [32]
  P_hat = W3b @ relu(W3a @ h + b3a) + b3b                       [32]

Sharding: every large matrix is row-sharded (output dim) across 8 cores;
W1 is replicated (tiny) so l1 needs no collective. The only collective is
one 16KB AllGather of h. The final 32-vector partials (W2b/W3b column
shards) are summed on the host during unsharding.

Layouts: activations live as "stationary" columns [128, nblk] so they can
be the matmul lhsT; weights are host-pre-transposed so W.T tiles stream
as the rhs. All biases are folded into the matmuls via an augmented
contraction element that is constant 1.

v2: GRU weights stream in fp8-e3m4 (one shared runtime scale for
w_ih/w_hh so gi+gh accumulate in a single PSUM bank; the inverse scale
is applied inside the gate activations). Weight streams are packed
gate-outer (r, n, z) so each gate's PSUM bank closes as early as
possible, and GRU weights are queued before the head weights so the
gate chain and the h all-gather sit right behind the GRU stream.
"""

import os
import sys

sys.path.insert(0, "/opt/trn_rl_repo")

import numpy as np
import ml_dtypes

# ---------------------------------------------------------------- constants
NCORES = 8
X_DIM = 32
IN2 = 128                      # l1 input dim
H1 = 1344                      # l1 output / GRU input dim
H1P = 1408                     # padded to 11*128 (pad block holds the bias row)
GH = 4096                      # GRU hidden
GHP = 4224                     # padded to 33*128 (aug block holds bias row)
H2 = 4096                      # head hidden
SH = 512                       # per-core hidden slice (GH/8 == H2/8)
K1 = H1P // 128                # 11 contraction blocks for gi
KH = GHP // 128                # 33 contraction blocks for gh / heads
KF = 640 // 128                # 5 contraction blocks for the final gemv

GRU_CHUNK = 11                 # k-blocks per DMA chunk for the fp8 GRU stream
HEAD_CHUNK = 11                # k-blocks per DMA chunk for w2at/w3at

E3M4_MAX = 15.0                # absmax target for the e3m4 weight scale

_GATHER = os.environ.get("KERNEL_GATHER", "bcast")
_GRU_DT = os.environ.get("KERNEL_GRU_DTYPE", "e3")

_compiled = {}


def _build(gather, gru_dt_name):
    import concourse.bass as bass  # noqa: F401
    import concourse.mybir as mybir
    import concourse.tile as tile
    from concourse import bacc

    F32 = mybir.dt.float32
    BF16 = mybir.dt.bfloat16
    GDT = {"e3": mybir.dt.float8e3, "bf16": BF16}[gru_dt_name]
    GBYTES = 1 if gru_dt_name == "e3" else 2
    AF = mybir.ActivationFunctionType
    ALU = mybir.AluOpType
    ts = bass.ts

    nc = bacc.Bacc("TRN2", target_bir_lowering=False, debug=False, num_devices=NCORES)

    # ------------------------------------------------------------- I/O decl
    xvec = nc.dram_tensor("xvec", [128, 1], BF16, kind="ExternalInput")
    w1t = nc.dram_tensor("w1t", [128, H1P], BF16, kind="ExternalInput")
    b1s = nc.dram_tensor("b1s", [128, K1], F32, kind="ExternalInput")
    # fp8 GRU stream, packed gate-outer (r, n, z); per gate: whh chunks
    # then the wih chunk, each [nkb*128*SH] per-partition contiguous.
    grup = nc.dram_tensor("grup", [3 * (KH + K1) * 128 * SH], GDT, kind="ExternalInput")
    w2ap = nc.dram_tensor("w2ap", [KH * 128 * SH], BF16, kind="ExternalInput")
    w3ap = nc.dram_tensor("w3ap", [KH * 128 * SH], BF16, kind="ExternalInput")
    h0stat = nc.dram_tensor("h0stat", [128, KH], BF16, kind="ExternalInput")
    h0row = nc.dram_tensor("h0row", [1, SH], F32, kind="ExternalInput")
    invs = nc.dram_tensor("invs", [1, 1], F32, kind="ExternalInput")
    w2bt = nc.dram_tensor("w2bt", [KF * 128, 32], BF16, kind="ExternalInput")
    w3bt = nc.dram_tensor("w3bt", [KF * 128, 32], BF16, kind="ExternalInput")
    ident = nc.dram_tensor("ident", [32, 128], F32, kind="ExternalInput")
    coreid = nc.dram_tensor("coreid", [1, 1], mybir.dt.uint32, kind="ExternalInput")
    out = nc.dram_tensor("out", [1, 64], F32, kind="ExternalOutput")

    # GRU stream chunk table: per gate phase g: 3 whh chunks + 1 wih chunk.
    # Each entry: (dram_off_elems, nkb, stat_kind, kb0, start, stop)
    gru_chunks = []
    off = 0
    for g in range(3):
        for ci in range(3):
            kb0 = ci * 11
            gru_chunks.append((off, 11, "h0", kb0, kb0 == 0, False))
            off += 11 * 128 * SH
        gru_chunks.append((off, K1, "l1", 0, False, True))
        off += K1 * 128 * SH
    assert off == 3 * (KH + K1) * 128 * SH

    head_chunks = []
    for t in (w2ap, w3ap):
        for kb0 in range(0, KH, HEAD_CHUNK):
            head_chunks.append((t, kb0, min(HEAD_CHUNK, KH - kb0)))

    with tile.TileContext(nc) as tc:
        with (
            tc.tile_pool(name="const", bufs=1) as cp,
            tc.tile_pool(name="gru", bufs=4) as gp,
            tc.tile_pool(name="head", bufs=4) as hp,
            tc.tile_pool(name="acts", bufs=1) as ap,
            tc.tile_pool(name="dram", bufs=1, space="DRAM") as dp,
        ):
            # -------------------------------- weight stream DMAs (sync queue)
            gru_tiles = []
            for off, nkb, stat_kind, kb0, st, sp in gru_chunks:
                g = gp.tile([128, GRU_CHUNK * SH], GDT, tag="gruw", name="gruw")
                sz = nkb * 128 * SH
                nc.sync.dma_start(
                    g[:, 0 : nkb * SH],
                    grup[off : off + sz].rearrange("(p x) -> p x", p=128),
                )
                gru_tiles.append(g)
            head_tiles = []
            for t, kb0, nkb in head_chunks:
                g = hp.tile([128, HEAD_CHUNK * SH], BF16, tag="headw", name="headw")
                o = kb0 * 128 * SH
                sz = nkb * 128 * SH
                nc.sync.dma_start(
                    g[:, 0 : nkb * SH],
                    t[o : o + sz].rearrange("(p x) -> p x", p=128),
                )
                head_tiles.append(g)

            # ------------------------------------- constant loads (scalar q)
            x_sb = cp.tile([128, 1], BF16, tag="x")
            nc.scalar.dma_start(x_sb[:], xvec[:])
            w1_sb = cp.tile([128, H1P], BF16, tag="w1")
            nc.scalar.dma_start(w1_sb[:], w1t[:])
            b1_sb = cp.tile([128, K1], F32, tag="b1")
            nc.scalar.dma_start(b1_sb[:], b1s[:])
            h0s_sb = cp.tile([128, KH], BF16, tag="h0s")
            nc.scalar.dma_start(h0s_sb[:], h0stat[:])
            h0r_sb = cp.tile([1, SH], F32, tag="h0r")
            nc.scalar.dma_start(h0r_sb[:], h0row[:])
            invs_sb = cp.tile([1, 1], F32, tag="invs")
            nc.scalar.dma_start(invs_sb[:], invs[:])
            id_sb = cp.tile([32, 128], F32, tag="id")
            nc.scalar.dma_start(id_sb[:], ident[:])
            cid_sb = cp.tile([1, 1], mybir.dt.uint32, tag="cid")
            nc.scalar.dma_start(cid_sb[:], coreid[:])
            # gather target: written remotely by all 8 cores' broadcasts.
            # memset early so the slot is reserved for the whole kernel and
            # cannot alias a streaming tile when a peer's write lands.
            h_sb = ap.tile([128, KH], BF16, tag="hstat")
            nc.vector.memset(h_sb[:], 0.0)
            w2b_sb = cp.tile([128, KF, 32], BF16, tag="w2b")
            nc.scalar.dma_start(
                w2b_sb[:], w2bt[:].rearrange("(k p) n -> p k n", p=128)
            )
            w3b_sb = cp.tile([128, KF, 32], BF16, tag="w3b")
            nc.scalar.dma_start(
                w3b_sb[:], w3bt[:].rearrange("(k p) n -> p k n", p=128)
            )

            # startup all-core rendezvous on gpsimd only: absorbs per-core
            # NEFF launch skew (tens of us) off the critical path, while the
            # sync ring streams weights unaffected. Also warms ncfw for the
            # real gather.
            bar_sb = cp.tile([1, 8], mybir.dt.uint32, tag="bar")
            nc.gpsimd.memset(bar_sb[:], 1)
            bar_in = dp.tile([1, 8], mybir.dt.uint32, name="bar_in")
            bar_out = dp.tile([1, 8], mybir.dt.uint32, name="bar_out")
            nc.gpsimd.dma_start(bar_in[:], bar_sb[:])
            nc.gpsimd.collective_compute(
                "AllReduce",
                mybir.AluOpType.add,
                replica_groups=[list(range(NCORES))],
                ins=[bar_in[:].opt()],
                outs=[bar_out[:].opt()],
            )
            bar2_sb = cp.tile([1, 8], mybir.dt.uint32, tag="bar2")
            nc.gpsimd.dma_start(bar2_sb[:], bar_out[:])

            with tc.tile_pool(name="psA", bufs=1, space="PSUM") as psA:
                # ------------------------------------------- L1 (W-stationary)
                l1p = psA.tile([128, K1], F32, tag="l1p")
                for j in range(K1):
                    nc.tensor.matmul(
                        l1p[:, j : j + 1],
                        w1_sb[:, ts(j, 128)],
                        x_sb[:],
                        start=True,
                        stop=True,
                    )
                l1t = ap.tile([128, K1], F32, tag="l1t")
                nc.vector.scalar_tensor_tensor(
                    l1t[:], l1p[:], 1.0, b1_sb[:], ALU.mult, ALU.add
                )
                l1_sb = ap.tile([128, K1], BF16, tag="l1s")
                nc.scalar.activation(l1_sb[:], l1t[:], AF.Relu)

                # --------------------------- GRU matmuls, gate-outer (r,n,z)
                # banks: A = gi_r+gh_r, D = gh_n, C = gi_n, B = gi_z+gh_z
                bankA = psA.tile([1, SH], F32, tag="bankA", name="bankA")
                bankD = psA.tile([1, SH], F32, tag="bankD", name="bankD")
                bankC = psA.tile([1, SH], F32, tag="bankC", name="bankC")
                bankB = psA.tile([1, SH], F32, tag="bankB", name="bankB")
                phase_banks = [(bankA, bankA), (bankD, bankC), (bankB, bankB)]
                inv = invs_sb[0:1, 0:1]

                r_t = ap.tile([1, SH], F32, tag="r")
                z_t = ap.tile([1, SH], F32, tag="z")
                n_t = ap.tile([1, SH], F32, tag="n")
                t_m = ap.tile([1, SH], F32, tag="gtmp", bufs=4)
                t_n = ap.tile([1, SH], F32, tag="gtmp", bufs=4)
                t_d = ap.tile([1, SH], F32, tag="gtmp", bufs=4)
                t_e = ap.tile([1, SH], F32, tag="gtmp", bufs=4)
                h_row = ap.tile([1, SH], F32, tag="hrow")

                for g in range(3):
                    hbank, lbank = phase_banks[g]
                    for ci in range(4):
                        off, nkb, stat_kind, kb0, st, sp = gru_chunks[g * 4 + ci]
                        dst = hbank if stat_kind == "h0" else lbank
                        stat = h0s_sb if stat_kind == "h0" else l1_sb
                        if g == 1:  # n gate: separate banks, own start/stop
                            st = kb0 == 0
                            sp = kb0 + nkb == (KH if stat_kind == "h0" else K1)
                        for kk in range(nkb):
                            kb = kb0 + kk
                            nc.tensor.matmul(
                                dst[:],
                                stat[:, kb : kb + 1],
                                gru_tiles[g * 4 + ci][:, kk * SH : (kk + 1) * SH],
                                start=(st and kk == 0),
                                stop=(sp and kk == nkb - 1),
                            )
                    # gate math interleaved with the next phase's stream
                    if g == 0:
                        nc.scalar.activation(r_t[:], bankA[:], AF.Sigmoid, scale=inv)
                    elif g == 1:
                        nc.vector.tensor_tensor(t_m[:], r_t[:], bankD[:], ALU.mult)
                        nc.vector.tensor_tensor(t_n[:], t_m[:], bankC[:], ALU.add)
                        nc.scalar.activation(n_t[:], t_n[:], AF.Tanh, scale=inv)
                        nc.vector.tensor_tensor(t_d[:], h0r_sb[:], n_t[:], ALU.subtract)
                    else:
                        nc.scalar.activation(z_t[:], bankB[:], AF.Sigmoid, scale=inv)
                        nc.vector.tensor_tensor(t_e[:], z_t[:], t_d[:], ALU.mult)
                        nc.vector.tensor_tensor(h_row[:], n_t[:], t_e[:], ALU.add)

            # ------------- h row -> stationary cols via rank-1 PE matmuls
            one = id_sb[0:1, 0:1]  # constant 1.0
            with tc.tile_pool(name="psB", bufs=1, space="PSUM") as psB:
                hT4 = psB.tile([128, 4], F32, tag="hT4")
                for k in range(4):
                    nc.tensor.matmul(
                        hT4[:, k : k + 1],
                        h_row[0:1, ts(k, 128)],
                        one,
                        start=True,
                        stop=True,
                    )
                hloc = ap.tile([128, 4], BF16, tag="hloc")
                nc.vector.tensor_copy(hloc[:], hT4[:])

                # ---------------- all-gather h across the 8 cores
                h_use = ap.tile([128, KH], BF16, tag="huse")
                if gather == "bcast":
                    psem = nc.alloc_semaphore("bc_prep_sem")
                    lsem = nc.alloc_semaphore("bc_local_sem")
                    rsem = nc.alloc_semaphore("bc_remote_sem")
                    with tc.tile_critical():
                        eng = nc.gpsimd
                        reg = eng.alloc_register("cid_reg")
                        # order after the startup alignment barrier
                        eng.reg_load(reg, bar2_sb[0:1, 0:1])
                        eng.reg_load(reg, cid_sb[0:1, 0:1])
                        for c in range(NCORES):
                            with eng.If_eq(reg, c):
                                eng.remote_dma_broadcast(
                                    out_ap=h_sb[:, c * 4 : (c + 1) * 4],
                                    in_ap=hloc[:],
                                    remote_sem=rsem,
                                    local_sem=lsem,
                                    rdests=[(0, k) for k in range(NCORES)],
                                ).then_inc(psem, 1)
                            with eng.Else():
                                eng.nop()
                        eng.wait_ge(psem, 1)
                        eng.trigger_dma(count=1)
                        eng.wait_ge(lsem, 16)
                        eng.wait_ge(rsem, 16)
                        eng.memset(h_sb[0:1, 32:33], 1.0)  # aug element
                        # copy into h_use so downstream consumers depend on
                        # the gathered data (remote writes invisible to Tile)
                        eng.tensor_copy(h_use[:], h_sb[:])
                else:
                    cc_in = dp.tile([128, 4], BF16, name="cc_in")
                    cc_out = dp.tile([NCORES, 128, 4], BF16, name="cc_out")
                    nc.scalar.dma_start(cc_in[:], hloc[:])
                    nc.gpsimd.collective_compute(
                        "AllGather",
                        mybir.AluOpType.bypass,
                        replica_groups=[list(range(NCORES))],
                        ins=[cc_in[:].opt()],
                        outs=[cc_out[:].opt()],
                    )
                    # cc_out[c, p, j] = h block col (c*4+j) partition p
                    nc.scalar.dma_start(
                        h_sb[:, 0:32].rearrange("p (c j) -> p c j", j=4),
                        cc_out[:].rearrange("c p j -> p c j"),
                    )
                    nc.vector.memset(h_sb[0:1, 32:33], 1.0)
                    nc.vector.tensor_copy(h_use[:], h_sb[:])

                # ------------------------------------------- head matmuls
                a2p = psB.tile([1, SH], F32, tag="a2p")
                a3p = psB.tile([1, SH], F32, tag="a3p")
                nh = len(head_chunks) // 2
                for ci, (t, kb0, nkb) in enumerate(head_chunks):
                    dst = a2p if ci < nh else a3p
                    for kk in range(nkb):
                        kb = kb0 + kk
                        nc.tensor.matmul(
                            dst[:],
                            h_use[:, kb : kb + 1],
                            head_tiles[ci][:, kk * SH : (kk + 1) * SH],
                            start=(kb == 0),
                            stop=(kb == KH - 1),
                        )

                a2row = ap.tile([1, SH], F32, tag="a2row")
                nc.scalar.activation(a2row[:], a2p[:], AF.Relu)
                a3row = ap.tile([1, SH], F32, tag="a3row")
                nc.scalar.activation(a3row[:], a3p[:], AF.Relu)

                # ---------------- a rows -> stationary cols (rank-1 PE)
                aT2 = psB.tile([128, 4], F32, tag="aT2")
                aT3 = psB.tile([128, 4], F32, tag="aT3")
                for k in range(4):
                    nc.tensor.matmul(
                        aT2[:, k : k + 1], a2row[0:1, ts(k, 128)], one,
                        start=True, stop=True,
                    )
                for k in range(4):
                    nc.tensor.matmul(
                        aT3[:, k : k + 1], a3row[0:1, ts(k, 128)], one,
                        start=True, stop=True,
                    )
                a_sb = ap.tile([128, 9], BF16, tag="astat")
                nc.vector.tensor_copy(a_sb[:, 0:4], aT2[:])
                nc.vector.tensor_copy(a_sb[:, 4:8], aT3[:])
                nc.vector.memset(a_sb[:, 8:9], 0.0)
                nc.vector.memset(a_sb[0:1, 8:9], 1.0)

                # ------------------------------------------- final gemvs
                op = psB.tile([1, 64], F32, tag="outp")
                cols2 = [0, 1, 2, 3, 8]
                cols3 = [4, 5, 6, 7, 8]
                for ki, k in enumerate(cols2):
                    nc.tensor.matmul(
                        op[:, 0:32],
                        a_sb[:, k : k + 1],
                        w2b_sb[:, ki, :],
                        start=(ki == 0),
                        stop=(ki == KF - 1),
                    )
                for ki, k in enumerate(cols3):
                    nc.tensor.matmul(
                        op[:, 32:64],
                        a_sb[:, k : k + 1],
                        w3b_sb[:, ki, :],
                        start=(ki == 0),
                        stop=(ki == KF - 1),
                    )
                out_sb = ap.tile([1, 64], F32, tag="osb")
                nc.scalar.activation(out_sb[:], op[:], AF.Copy)
                nc.gpsimd.dma_start(out[:], out_sb[:])

    nc.compile()
    return nc


def _get_nc():
    key = (_GATHER, _GRU_DT)
    if key not in _compiled:
        _compiled[key] = _build(*key)
    return _compiled[key]


# ------------------------------------------------------------------ host prep
def _prep_in_maps(inputs):
    f32 = np.float32
    bf16 = ml_dtypes.bfloat16
    gnp = {"e3": ml_dtypes.float8_e3m4, "bf16": bf16}[_GRU_DT]

    x = np.concatenate(
        [
            np.asarray(inputs[k], dtype=f32).ravel()
            for k in ("state_inno", "precov", "residual", "meas_cov")
        ]
    )
    W1 = np.asarray(inputs["W1"], f32)
    b1 = np.asarray(inputs["b1"], f32)
    w_ih = np.asarray(inputs["w_ih"], f32)
    w_hh = np.asarray(inputs["w_hh"], f32)
    b_ih = np.asarray(inputs["b_ih"], f32)
    b_hh = np.asarray(inputs["b_hh"], f32)
    h0 = np.asarray(inputs["h0"], f32)
    W2a = np.asarray(inputs["W2a"], f32)
    b2a = np.asarray(inputs["b2a"], f32)
    W2b = np.asarray(inputs["W2b"], f32)
    b2b = np.asarray(inputs["b2b"], f32)
    W3a = np.asarray(inputs["W3a"], f32)
    b3a = np.asarray(inputs["b3a"], f32)
    W3b = np.asarray(inputs["W3b"], f32)
    b3b = np.asarray(inputs["b3b"], f32)

    # shared e3m4 scale for w_ih/w_hh (+ their biases): gi and gh must
    # accumulate in the same PSUM bank, so one scale covers both.
    if _GRU_DT == "e3":
        absmax = max(
            np.abs(w_ih).max(), np.abs(w_hh).max(),
            np.abs(b_ih).max(), np.abs(b_hh).max(), 1e-30,
        )
        s_g = E3M4_MAX / float(absmax)
    else:
        s_g = 1.0
    inv_s = np.array([[1.0 / s_g]], dtype=f32)

    # shared (core-independent) tensors
    xvec = x.astype(bf16).reshape(128, 1)
    w1t = np.zeros((128, H1P), f32)
    w1t[:, :H1] = W1.T
    w1t = w1t.astype(bf16)
    b1s = np.zeros((128, K1), f32)
    b1pad = np.zeros(H1P, f32)
    b1pad[:H1] = b1
    b1pad[H1] = 1.0  # aug element: relu(0 + 1) = 1 feeds the bias rows of gi
    b1s[:, :] = b1pad.reshape(K1, 128).T
    h0stat = np.zeros((128, KH), f32)
    h0stat[:, :32] = h0.reshape(32, 128).T
    h0stat[0, 32] = 1.0
    h0stat = h0stat.astype(bf16)
    identity = np.zeros((32, 128), dtype=f32)
    identity[:, :32] = np.eye(32, dtype=f32)

    wihT = w_ih.T  # [H1, 3GH]
    whhT = w_hh.T  # [GH, 3GH]
    W2aT = W2a.T  # [GH, H2]
    W3aT = W3a.T

    def pack_stream(mat, nkb_total, step):
        width = mat.shape[1]
        blocks = []
        for kb0 in range(0, nkb_total, step):
            nkb = min(step, nkb_total - kb0)
            blk = (
                mat[kb0 * 128 : (kb0 + nkb) * 128, :]
                .reshape(nkb, 128, width)
                .transpose(1, 0, 2)
                .reshape(-1)
            )
            blocks.append(blk)
        return np.concatenate(blocks)

    in_maps = []
    for c in range(NCORES):
        s = slice(c * SH, (c + 1) * SH)

        # gate-outer fp8 GRU stream: phases r(0), n(2), z(1); per phase
        # 3 whh chunks of 11 kb then 1 wih chunk of 11 kb.
        parts = []
        for gate in (0, 2, 1):
            gs = slice(gate * GH + c * SH, gate * GH + (c + 1) * SH)
            whh_g = np.zeros((GHP, SH), f32)
            whh_g[:GH, :] = whhT[:, gs] * s_g
            whh_g[GH, :] = b_hh[gs] * s_g
            wih_g = np.zeros((H1P, SH), f32)
            wih_g[:H1, :] = wihT[:, gs] * s_g
            wih_g[H1, :] = b_ih[gs] * s_g
            parts.append(pack_stream(whh_g, KH, GRU_CHUNK))
            parts.append(pack_stream(wih_g, K1, GRU_CHUNK))
        grup = np.concatenate(parts).astype(gnp)

        w2at = np.zeros((GHP, SH), f32)
        w2at[:GH, :] = W2aT[:, s]
        w2at[GH, :] = b2a[s]
        w3at = np.zeros((GHP, SH), f32)
        w3at[:GH, :] = W3aT[:, s]
        w3at[GH, :] = b3a[s]
        w2ap = pack_stream(w2at, KH, HEAD_CHUNK)
        w3ap = pack_stream(w3at, KH, HEAD_CHUNK)
        w2bt = np.zeros((KF * 128, 32), f32)
        w2bt[:SH, :] = W2b[:, s].T
        w3bt = np.zeros((KF * 128, 32), f32)
        w3bt[:SH, :] = W3b[:, s].T
        if c == 0:
            w2bt[SH, :] = b2b
            w3bt[SH, :] = b3b

        in_maps.append(
            {
                "xvec": xvec,
                "w1t": w1t,
                "b1s": b1s,
                "grup": grup,
                "h0stat": h0stat,
                "h0row": h0[s].reshape(1, SH),
                "invs": inv_s,
                "w2ap": w2ap.astype(bf16),
                "w3ap": w3ap.astype(bf16),
                "w2bt": w2bt.astype(bf16),
                "w3bt": w3bt.astype(bf16),
                "ident": identity,
                "coreid": np.array([[c]], dtype=np.uint32),
            }
        )
    return in_maps


def run(inputs, trace=False):
    from concourse.bass_utils import run_bass_kernel_spmd

    nc = _get_nc()
    in_maps = _prep_in_maps(inputs)
    res = run_bass_kernel_spmd(
        nc, in_maps, core_ids=list(range(NCORES)), trace=trace
    )
    total = np.sum([np.asarray(r["out"], np.float64) for r in res.results], axis=0)
    total = total.astype(np.float32).ravel()
    x_hat = total[:32].reshape(X_DIM, 1)
    P_hat = total[32:].reshape(X_DIM, 1)
    return (x_hat, P_hat), res


def kernel(**inputs):
    (x_hat, P_hat), _ = run(inputs, trace=False)
    return (x_hat, P_hat)
